# revision 23
# baseline (speedup 1.0000x reference)
import sys
from contextlib import ExitStack

import numpy as np
import ml_dtypes

sys.path.insert(0, "/opt/trn_rl_repo")

try:
    import jax
    jax.config.update("jax_compilation_cache_dir", "/tmp/jax_cc_cache")
    jax.config.update("jax_persistent_cache_min_compile_time_secs", 0.0)
    jax.config.update("jax_persistent_cache_min_entry_size_bytes", 0)
except Exception:
    pass

import concourse.bass as bass
import concourse.tile as tile
from concourse import bacc, mybir
from concourse.bass_utils import run_bass_kernel_spmd

B, H, W, CH = 4, 80, 80, 256
NCLS, DIM = 22, 256
ROWS = 40                 # rows per core
NPIX = ROWS * W           # 3200 output pixels per core
NT = (ROWS + 2) * W + 2   # 3362 strip positions (halo rows + end pad)
G = 512                   # pixel chunk
CHUNKS = [(g, min(G, NPIX - g)) for g in range(0, NPIX, G)]  # 6x512 + 128
NCH = len(CHUNKS)
SELC = 9 * NPIX           # 28800 sel columns (chunk-major, k-major, pixel)
WC = 36 * 128             # 4608 weight cols: block dh*18+k*2+h of [128c x 128d]
# per-chunk x piece boundaries: chunk ci reads strip cols < g0+g+163
XPB = [0] + [min(g0 + g + 163, NT) for g0, g in CHUNKS]
F32 = mybir.dt.float32
F16 = mybir.dt.float16
BF16 = mybir.dt.bfloat16
BF16NP = ml_dtypes.bfloat16


def _build_nc():
    nc = bacc.Bacc("TRN2", target_bir_lowering=False, debug=False,
                   enable_asserts=False, num_devices=8)
    xa_d = nc.dram_tensor("xa", [128, 2 * NT], BF16, kind="ExternalInput").ap()
    sel_d = nc.dram_tensor("selb", [128, SELC], BF16, kind="ExternalInput").ap()
    wt_d = nc.dram_tensor("wt", [128, WC], BF16, kind="ExternalInput").ap()
    out_d = nc.dram_tensor("out", [2 * 128, NPIX], F16, kind="ExternalOutput").ap()

    sel_off = [0]
    for _, g in CHUNKS:
        sel_off.append(sel_off[-1] + 9 * g)

    with tile.TileContext(nc) as tc, ExitStack() as ctx:
        xp = ctx.enter_context(tc.tile_pool(name="xp", bufs=1))
        xgp = ctx.enter_context(tc.tile_pool(name="xgp", bufs=3))
        outp = ctx.enter_context(tc.tile_pool(name="outp", bufs=4))
        zp = ctx.enter_context(tc.tile_pool(name="zp", bufs=4, space="PSUM"))

        xa_t = xp.tile([128, 2 * NT], BF16)
        xb_t = xp.tile([128, 2 * NT], BF16)
        wt_t = xp.tile([128, WC], BF16)
        S_t = xp.tile([128, SELC], BF16)
        warm = xp.tile([128, G], BF16)

        # PE warmup: ramp the clock to max p-state while inputs stream in
        nc.gpsimd.memset(warm[:], 0.0)
        zw = zp.tile([128, G], F32)
        for _ in range(7):
            nc.tensor.matmul(zw[:], warm[:, :128], warm[:], start=True,
                             stop=True)

        # ---- ALL input DMAs on the single SP queue in exact need order ----
        # (one FIFO == strict priority; descriptor round-robin across queues
        # is byte-unfair and starves the critical head-of-line pieces)
        def xapiece(ci):
            a, b = XPB[ci], XPB[ci + 1]
            for h in range(2):
                nc.sync.dma_start(xa_t[:, h * NT + a:h * NT + b],
                                  xa_d[:, h * NT + a:h * NT + b])

        def spiece(a, b):
            nc.sync.dma_start(S_t[:, a:b], sel_d[:, a:b])

        def wpiece(a, b):
            nc.sync.dma_start(wt_t[:, a:b], wt_d[:, a:b])

        spiece(0, 512)           # S chunk0 k0
        xapiece(0)
        wpiece(0, 512)           # w blocks 0-3 (dh0 chain head)
        spiece(512, 2560)        # S chunk0 k1-k4
        wpiece(512, 2304)        # rest of dh0 chain blocks
        spiece(2560, 4608)       # S chunk0 k5-k8
        wpiece(2304, WC)         # dh1 chain blocks
        xapiece(1)
        a, b = sel_off[1], sel_off[2]
        th = (b - a) // 3
        spiece(a, a + th)
        spiece(a + th, a + 2 * th)
        spiece(a + 2 * th, b)
        for ci in range(2, NCH):
            xapiece(ci)
            spiece(sel_off[ci], sel_off[ci + 1])

        # DVE derives the 1-shifted x copy (xb[m] = xa[m+1]) per piece,
        # interleaved with the gating stream (fast 2-port copy mode);
        # pieces overlap 1 col so the union is gapless up to XPB[ci+1]-1
        def xbcopy(ci, h):
            a, b = max(0, XPB[ci] - 2), XPB[ci + 1] - 1
            nc.vector.tensor_scalar_mul(xb_t[:, h * NT + a:h * NT + b],
                                        xa_t[:, h * NT + a + 1:h * NT + b + 1],
                                        1.0)

        # ---- main pipeline ----
        for ci, (g0, g) in enumerate(CHUNKS):
            so = sel_off[ci]
            xg = xgp.tile([128, 18 * G], BF16)
            for k in range(9):
                di, dj = k // 3, k % 3
                for h in range(2):
                    if ci == 0 and k == 1:
                        # xb chunk-0 pieces just before their first use
                        xbcopy(0, h)
                    if dj == 1:
                        src = xb_t[:, h * NT + g0 + 80 * di:
                                   h * NT + g0 + 80 * di + g]
                    else:
                        src = xa_t[:, h * NT + g0 + 80 * di + dj:
                                   h * NT + g0 + 80 * di + dj + g]
                    nc.vector.tensor_mul(xg[:, (2 * k + h) * g:(2 * k + h + 1) * g],
                                         src, S_t[:, so + k * g:so + (k + 1) * g])
            if ci + 1 < NCH:
                xbcopy(ci + 1, 0)
                xbcopy(ci + 1, 1)

            for dh in range(2):
                z = zp.tile([128, G], F32)
                for k in range(9):
                    for h in range(2):
                        blk = dh * 18 + k * 2 + h
                        nc.tensor.matmul(
                            z[:, :g],
                            wt_t[:, blk * 128:(blk + 1) * 128],
                            xg[:, (2 * k + h) * g:(2 * k + h + 1) * g],
                            start=(k == 0 and h == 0), stop=(k == 8 and h == 1))
                o = outp.tile([128, G], F16)
                nc.scalar.copy(o[:, :g], z[:, :g])
                nc.scalar.dma_start(out_d[dh * 128:(dh + 1) * 128, g0:g0 + g],
                                    o[:, :g])
    nc.compile()
    return nc


_NC_CACHE = None


def _get_nc():
    global _NC_CACHE
    if _NC_CACHE is None:
        _NC_CACHE = _build_nc()
    return _NC_CACHE


def _prep_core(x, seg_mask, core):
    b, r0 = core // 2, 40 * (core % 2)
    xpad = np.pad(x[b], ((1, 1), (0, 0), (0, 0)))        # [82,80,256]
    strip = xpad[r0:r0 + 42].reshape(42 * W, CH)         # [3360,256]
    sp = np.zeros((NT, CH), np.float32)
    sp[1:1 + 42 * W] = strip
    A = sp.T                                             # [256, NT]
    xa = np.ascontiguousarray(
        np.concatenate([A[:128], A[128:]], axis=1)).astype(BF16NP)

    pads = np.pad(seg_mask[b], ((1, 1), (1, 1), (0, 0)))  # [82,82,22]
    mc = seg_mask[b][r0:r0 + 40]                          # [40,80,22]
    smax = mc.max(-1, keepdims=True)
    eq = (mc == smax).astype(np.float32)
    sel = np.empty((40, 80, 9), np.float32)
    for k in range(9):
        di, dj = k // 3 - 1, k % 3 - 1
        sel[..., k] = (eq * pads[r0 + 1 + di:r0 + 41 + di,
                                 1 + dj:81 + dj]).sum(-1)
    cnt = (sel != 0).astype(np.float32).sum(-1, keepdims=True)
    selp = (sel * (9.0 / np.maximum(cnt, 1.0))).reshape(NPIX, 9)
    flat = np.concatenate(
        [selp[g0:g0 + g].T.reshape(-1) for g0, g in CHUNKS]).astype(BF16NP)
    selb = np.ascontiguousarray(np.broadcast_to(flat[None, :], (128, SELC)))
    return xa, selb


def _prep_in_maps(x, seg_mask, conv_w):
    w9 = conv_w.reshape(CH, 9, DIM)
    wt = np.empty((128, WC), np.float32)
    for k in range(9):
        for h in range(2):
            for dh in range(2):
                blk = dh * 18 + k * 2 + h
                wt[:, blk * 128:(blk + 1) * 128] = \
                    w9[128 * h:128 * (h + 1), k, dh * 128:(dh + 1) * 128]
    wt = np.ascontiguousarray(wt).astype(BF16NP)

    in_maps = []
    for core in range(8):
        xa, selb = _prep_core(x, seg_mask, core)
        in_maps.append({"xa": xa, "selb": selb, "wt": wt})
    return in_maps


def kernel(x, seg_mask, conv_w):
    x = np.asarray(x, np.float32)
    seg_mask = np.asarray(seg_mask, np.float32)
    conv_w = np.asarray(conv_w, np.float32)

    in_maps = _prep_in_maps(x, seg_mask, conv_w)
    nc = _get_nc()
    res = run_bass_kernel_spmd(nc, in_maps, core_ids=list(range(8)))

    out = np.empty((B, H, W, DIM), np.float32)
    for core in range(8):
        b, r0 = core // 2, 40 * (core % 2)
        out[b, r0:r0 + 40] = res.results[core]["out"].astype(
            np.float32).T.reshape(ROWS, W, DIM)
    return out
